# revision 14
# baseline (speedup 1.0000x reference)
"""Trainium2 Bass kernel for the BN-attention module (nn_Attention).

Full inputs -> full output. Sharding: 8 cores = (batch b in 0..3) x
(head-group g in 0..1, 4 heads each). Each core computes its batch's
4-head attention and a partial output projection; the host sums the two
head-group partials per batch and adds the projection BN bias.

Numerics: BN scales are folded into the weights on the host. The Q/K
path runs fp16; exp output (pe) is bf16. Most exp tiles run on the
ScalarE (true exp); a few per chunk run on the VectorE as a Schraudolph
fast-exp: the bf16 bit pattern round(S*128/ln2 + 16256) is written via
an int16-bitcast tensor_scalar, giving 2^(S*log2 e) with ~3% max
multiplicative error that largely cancels in the softmax ratio
(measured end-to-end rel err ~1e-2 vs the 2e-2 budget).

Layout: attention is transposed, S^T = K^T Q with keys (m) on
partitions. All 4 heads are packed in one m-tile step: the four 32-row
K stationaries occupy the four PE row groups (tile_position), so the
four 256-col S^T matmuls run concurrently, and one [128,1024] exp tile
covers all heads. Chunks are 256 query columns (9 chunks); per-head
PV accumulators share PSUM banks in pairs, so ST double-buffering (4
banks) + xx (2) + finish/proj staging (2) exactly fill the 8 banks.

Softmax denominators: exp tiles are pair-summed (t-level) and
pair-pair-summed (u-level) on the VectorE/GpSimd (bf16), and the
folded tiles are column-summed by ones-matmuls at the chunk boundary
(filling the PE's wait on the trailing exp). The finish (reciprocal,
GPSIMD partition_broadcast, normalize, fp16 bias+relu) and the output
projection are deferred into the next chunk's loop so the PE never
stalls at chunk boundaries.
"""

import numpy as np

import concourse.bacc as bacc
import concourse.mybir as mybir
import concourse.tile as tile
from concourse.bass_utils import run_bass_kernel_spmd

# Problem dims (hardcoded per the spec)
B, C, H, W = 4, 256, 48, 48
N = H * W            # 2304
KD, NH, AR = 32, 8, 4
D = AR * KD          # 128 value dims per head
NHKD = NH * KD       # 256
DH = NH * D          # 1024
EPS = 1e-5

NHG = 4              # heads per core
J = 256              # consts tile width (legacy host layout)
MT = 128             # m-tile (key tile)
NMT = N // MT        # 18
JW = 256             # query-chunk width
NCH = N // JW        # 9 chunks

F32 = mybir.dt.float32
F32R = mybir.dt.float32r
BF16 = mybir.dt.bfloat16
FP16 = mybir.dt.float16
I16 = mybir.dt.int16
AF = mybir.ActivationFunctionType
OP = mybir.AluOpType

# Schraudolph bf16 fast-exp constants: bits = round(x*128/ln2 + 16256)
FE_A = float(128.0 / np.log(2.0))
FE_B = 16256.0

# chunks of 2304 by <=512 for the q/k projection matmuls
CHUNKS_512 = [(off, min(512, N - off)) for off in range(0, N, 512)]

# engine assignment knobs
DVE_EXP = {2, 6, 10, 14, 16}     # m-tiles whose exp runs as DVE fast-exp
GPS_T = {2, 4, 6, 8}             # t-level adds routed to GpSimd
DVE_EXP_C0 = set()               # chunk 0: DVE busy with v copies
GPS_T_C0 = {1, 3, 5, 7, 8}

_CACHE = {}


def _build_program():
    nc = bacc.Bacc("TRN2", target_bir_lowering=False, debug=False)

    x_in = nc.dram_tensor("x_in", [C, N], FP16, kind="ExternalInput")
    wqt_d = nc.dram_tensor("wqt", [C, 128], FP16, kind="ExternalInput")
    wkt_d = nc.dram_tensor("wkt", [C, 128], FP16, kind="ExternalInput")
    wvt_d = nc.dram_tensor("wvt", [C, 512], FP16, kind="ExternalInput")
    wpt_d = nc.dram_tensor("wpt", [512, C], FP16, kind="ExternalInput")
    bq_d = nc.dram_tensor("bq", [128, 1], F32, kind="ExternalInput")
    bk_d = nc.dram_tensor("bk", [128, 1], F32, kind="ExternalInput")
    bv_d = nc.dram_tensor("bv", [512, 1], F32, kind="ExternalInput")
    consts_d = nc.dram_tensor("consts", [128, 128 + J], F32R, kind="ExternalInput")
    ones_bf_d = nc.dram_tensor("ones_bf", [128, 1], BF16, kind="ExternalInput")
    out_d = nc.dram_tensor("outp", [C, N], F32, kind="ExternalOutput")

    with tile.TileContext(nc) as tc:
        with nc.allow_low_precision(reason="16-bit matmul rounding is intentional"), \
             tc.tile_pool(name="const", bufs=1) as constp, \
             tc.tile_pool(name="qk", bufs=1) as qkp, \
             tc.tile_pool(name="vt", bufs=1) as vtp, \
             tc.tile_pool(name="pexp", bufs=1) as pexpp, \
             tc.tile_pool(name="rp", bufs=1) as rp, \
             tc.tile_pool(name="work", bufs=2) as workp:

            # ---------- constants / inputs ----------
            # DMA order matters: x chunk 0 (the big transfer the first
            # projections need) leads the sync ring while the q/k weights +
            # biases go down the scalar ring in parallel.
            xf = [constp.tile([128, N], FP16, name=f"xf{c2}", tag=f"xf{c2}")
                  for c2 in range(2)]
            wqt, wkt, wvt = [], [], []
            nc.sync.dma_start(xf[0][:, 0:512], x_in.ap()[0:128, 0:512])
            for c2 in range(2):
                sl = slice(128 * c2, 128 * (c2 + 1))
                t = constp.tile([128, 128], FP16, name=f"wqt{c2}", tag=f"wqt{c2}")
                nc.scalar.dma_start(t[:], wqt_d.ap()[sl, :])
                wqt.append(t)
                t = constp.tile([128, 128], FP16, name=f"wkt{c2}", tag=f"wkt{c2}")
                nc.scalar.dma_start(t[:], wkt_d.ap()[sl, :])
                wkt.append(t)
            nc.sync.dma_start(xf[1][:, 0:512], x_in.ap()[128:256, 0:512])
            bq_t = constp.tile([128, 1], F32, name="bq_t", tag="bq_t")
            nc.scalar.dma_start(bq_t[:], bq_d.ap())
            bk_t = constp.tile([128, 1], F32, name="bk_t", tag="bk_t")
            nc.scalar.dma_start(bk_t[:], bk_d.ap())
            for c2 in range(2):
                t = constp.tile([128, 512], FP16, name=f"wvt{c2}",
                                tag=f"wvt{c2}")
                eng = nc.sync if c2 == 0 else nc.scalar
                eng.dma_start(t[:], wvt_d.ap()[128 * c2:128 * (c2 + 1), :])
                wvt.append(t)
            for off, w in CHUNKS_512:
                if off == 0:
                    continue
                nc.sync.dma_start(xf[0][:, off:off + w],
                                  x_in.ap()[0:128, off:off + w])
                nc.scalar.dma_start(xf[1][:, off:off + w],
                                    x_in.ap()[128:256, off:off + w])
            wpt = []
            for h in range(NHG):
                t = constp.tile([128, C], FP16, name=f"wpt{h}", tag=f"wpt{h}")
                eng = nc.sync if h % 2 == 0 else nc.scalar
                eng.dma_start(t[:], wpt_d.ap()[128 * h:128 * (h + 1), :])
                wpt.append(t)
            bv_t = []
            for h in range(NHG):
                t = constp.tile([128, 1], F32, name=f"bv{h}", tag=f"bv{h}")
                nc.sync.dma_start(t[:], bv_d.ap()[128 * h:128 * (h + 1), :])
                bv_t.append(t)
            ones_bf = constp.tile([128, 1], BF16, name="ones_bf", tag="ones_bf")
            nc.sync.dma_start(ones_bf[:], ones_bf_d.ap())

            # q/k live in lo (heads 0,1) and hi (heads 2,3) tiles, both on
            # partitions 0:64, so heads 2,3 reuse PE row groups 0,1: the
            # hardware serializes them after heads 0,1 (row conflict), and
            # the concurrent pair always targets two different PSUM banks.
            q_lo = qkp.tile([64, N], FP16, name="q_lo", tag="q_lo")
            q_hi = qkp.tile([64, N], FP16, name="q_hi", tag="q_hi")
            k_lo = qkp.tile([64, N], FP16, name="k_lo", tag="k_lo")
            k_hi = qkp.tile([64, N], FP16, name="k_hi", tag="k_hi")
            vt_all = vtp.tile([128, NMT * 512], BF16, name="vt_all", tag="vt_all")
            # st/pe column of each head: concurrent pairs (0,1) then (2,3)
            # land in different banks; bank pairing is (0,2) and (1,3)
            COLMAP = {0: 0, 1: 512, 2: 256, 3: 768}

            # ---------- phase 1: q piece 0 + k piece 0 only ----------
            # Just enough to start chunk 0's attention; remaining q/k
            # pieces and all v projections are deferred into the chunk
            # 0/1 loops so the first exp starts as early as possible.
            def qk_bias_split(lo, hi, ps, bias, qo, qw):
                nc.vector.tensor_scalar_add(lo[:, qo:qo + qw],
                                            ps[0:64, 0:qw], bias[0:64, :])
                nc.vector.tensor_scalar_add(hi[:, qo:qo + qw],
                                            ps[64:128, 0:qw], bias[64:128, :])

            with tc.tile_pool(name="p1", bufs=4, space="PSUM") as p1:
                ps = p1.tile([128, 512], F32, name="qproj", tag="p1")
                for c2 in range(2):
                    nc.tensor.matmul(ps[:], wqt[c2][:], xf[c2][:, 0:512],
                                     start=(c2 == 0), stop=(c2 == 1))
                qk_bias_split(q_lo, q_hi, ps, bq_t, 0, 512)
                ps = p1.tile([128, 512], F32, name="kproj", tag="p1")
                for c2 in range(2):
                    nc.tensor.matmul(ps[:], wkt[c2][:], xf[c2][:, 0:512],
                                     start=(c2 == 0), stop=(c2 == 1))
                qk_bias_split(k_lo, k_hi, ps, bk_t, 0, 512)

            # ---------- phase 2: attention + output projection ----------
            # Per chunk (256 query cols), loop 18 m-tiles: the four heads'
            # S^T matmuls land in the four row groups of the PE and fill
            # one [128,1024] psum tile; one exp evicts it to bf16 (ScalarE
            # true exp or VectorE Schraudolph per DVE_EXP). PV runs two
            # steps behind. Exp tiles pair-sum (t-level, DVE/GPS) and
            # pair-pair-sum (u-level), and u0..u3 get ones-matmul
            # column-sums at the chunk boundary; t8 joins in the deferred
            # finish.
            with tc.tile_pool(name="stp", bufs=1, space="PSUM") as stp, \
                 tc.tile_pool(name="xxp", bufs=1, space="PSUM") as xxp, \
                 tc.tile_pool(name="finp", bufs=1, space="PSUM") as finp:
                def emit_proj(joff_p, r_p):
                    # output projection over all four heads of a finished
                    # chunk (deferred into the next chunk's loop)
                    for ct in range(2):
                        op_ps = finp.tile([128, 256], F32, name="op_ps",
                                          tag=f"f{ct}")
                        for h in range(NHG):
                            nc.tensor.matmul(
                                op_ps[:, 0:JW],
                                wpt[h][:, 128 * ct:128 * (ct + 1)],
                                r_p[h],
                                start=(h == 0), stop=(h == NHG - 1))
                        o_sb = workp.tile([128, 256], F32, name="o_sb",
                                          tag="o_sb")
                        nc.vector.tensor_copy(o_sb[:, 0:JW],
                                              op_ps[:, 0:JW])
                        nc.sync.dma_start(
                            out_d.ap()[128 * ct:128 * (ct + 1),
                                       joff_p:joff_p + JW],
                            o_sb[:, 0:JW])

                # q/k piece schedules (pieces are 512 cols of q_all/k_all).
                # ST(mt) of chunk c reads k cols 128mt, so k piece p lands
                # by m-tile 4p of chunk 0; q piece p is needed by chunk 2p.
                KSCHED = {1: 1, 4: 2, 8: 3, 12: 4}    # chunk 0
                QSCHED0 = {5: 1}                      # chunk 0
                QSCHED1 = {1: 2, 7: 3, 13: 4}         # chunk 1

                pending = None     # (joff, r_ts) of the previous chunk
                fin_prev = None    # previous chunk's deferred finish closure
                for ci in range(NCH):
                    joff = JW * ci
                    r_ts = [None] * NHG
                    dve_exp = DVE_EXP_C0 if ci == 0 else DVE_EXP
                    gps_t = GPS_T_C0 if ci == 0 else GPS_T
                    # xx bank pairing matches the st/pe column pairing:
                    # bank 0 holds heads 0 (cols 0:256) and 2 (256:512),
                    # bank 1 holds heads 1 and 3.
                    xxt = [xxp.tile([128, 512], F32, name=f"xx{j}",
                                    tag=f"xx{j}") for j in range(2)]
                    xx = {}
                    for h in range(NHG):
                        xx[h] = xxt[h % 2][:, 256 * (h // 2):256 * (h // 2) + JW]
                    pexp = [None] * NMT
                    tsum = [None] * (NMT // 2)
                    usum = [None] * 4

                    def emit_qk_exp(mt):
                        # heads 0,1 run concurrently in row groups 0,1
                        # (banks 0,1); heads 2,3 reuse the same row groups
                        # so they serialize behind 0,1 while targeting the
                        # other half of each bank. One exp evicts the tile.
                        st = stp.tile([128, 1024], F32, name="st",
                                      tag=f"st{mt % 2}")
                        moff = 128 * mt
                        for h in range(NHG):
                            ksrc = k_lo if h < 2 else k_hi
                            qsrc = q_lo if h < 2 else q_hi
                            r0 = 32 * (h % 2)
                            nc.tensor.matmul(
                                st[:, COLMAP[h]:COLMAP[h] + JW],
                                ksrc[r0:r0 + 32, moff:moff + 128],
                                qsrc[r0:r0 + 32, joff:joff + JW],
                                start=True, stop=True,
                                tile_position=(r0, 0))
                        pe = pexpp.tile([128, 1024], BF16, name="pe",
                                        tag="pe", bufs=8)
                        if mt in dve_exp:
                            nc.vector.tensor_scalar(
                                pe[:].bitcast(I16), st[:],
                                FE_A, FE_B, OP.mult, OP.add)
                        else:
                            nc.scalar.activation(pe[:], st[:], AF.Exp)
                        pexp[mt] = pe

                    def emit_pv(mt, first=False):
                        # PV(1) executes first (psum reset); PV(0) joins
                        # late so the new chunk's xx reset never waits on
                        # the previous chunk's finish chain reading xx.
                        pe = pexp[mt]
                        for h in range(NHG):
                            # start=True clears the whole PSUM bank's
                            # has_written bits, so only the first head per
                            # shared bank may carry it; the second head's
                            # first write lands on cleared elements and
                            # overwrites.
                            nc.tensor.matmul(
                                xx[h],
                                vt_all[:, 512 * mt + 128 * h:
                                       512 * mt + 128 * (h + 1)],
                                pe[:, COLMAP[h]:COLMAP[h] + JW],
                                start=(first and h < 2),
                                stop=(mt == NMT - 1),
                                skip_group_check=True)

                    def emit_tree(k):
                        # u-level folds as soon as both t inputs are ready;
                        # u0..u3 cover m-tiles 0..15 and get their
                        # ones-matmuls at the chunk boundary.
                        if k % 2 == 1 and k < 8:
                            u = k // 2
                            t = workp.tile([128, 1024], BF16,
                                           name=f"u{u}", tag=f"u{u}",
                                           bufs=1)
                            nc.vector.tensor_tensor(
                                t[:], tsum[k - 1][:], tsum[k][:], OP.add)
                            usum[u] = t

                    # PV schedule: PVs trail their exp by two steps; the
                    # first PVs start later still (xx psum reset must wait
                    # the previous chunk's finish chain to release xx),
                    # with a 2-per-step catch-up ramp.
                    FS = 4
                    pv_order = [1, 2, 0] + list(range(3, NMT))
                    pv_sched = {}
                    done = 0
                    for _mt in range(FS, NMT):
                        target = min(NMT, _mt - 1)
                        n = min(2 if _mt > FS else 1,
                                max(0, target - done))
                        if n:
                            pv_sched[_mt] = pv_order[done:done + n]
                            done += n
                    pv_left = pv_order[done:]

                    sums_hs = [None, None]
                    for mt in range(NMT):
                        if mt == 0 and fin_prev is not None:
                            # the previous chunk's finish is ready now;
                            # emitting it ahead of ST(0) fills the PE's
                            # wait on the previous chunk's trailing exp
                            fin_prev()
                            fin_prev = None
                        emit_qk_exp(mt)
                        if ci == 0:
                            # v projection for m-tile mt+1, ahead of its PV
                            # consumer (v(0) handled at mt 0)
                            vmt = 0 if mt == 0 else mt + 1
                            if vmt <= NMT - 1:
                                ps_v = finp.tile([128, 512], F32, name="vps",
                                                 tag=f"f{(vmt + 1) % 2}")
                                for c2 in range(2):
                                    nc.tensor.matmul(
                                        ps_v[:],
                                        xf[c2][:, 128 * vmt:128 * (vmt + 1)],
                                        wvt[c2][:],
                                        start=(c2 == 0), stop=(c2 == 1))
                                nc.vector.tensor_copy(
                                    vt_all[:, 512 * vmt:512 * (vmt + 1)],
                                    ps_v[:])
                            if mt == 1:
                                # v(1) right behind v(0) to stay ahead of PV
                                ps_v = finp.tile([128, 512], F32, name="vps",
                                                 tag=f"f{0}")
                                for c2 in range(2):
                                    nc.tensor.matmul(
                                        ps_v[:],
                                        xf[c2][:, 128:256],
                                        wvt[c2][:],
                                        start=(c2 == 0), stop=(c2 == 1))
                                nc.vector.tensor_copy(
                                    vt_all[:, 512:1024], ps_v[:])
                        if ci <= 1:
                            sched = (KSCHED if ci == 0 else {})
                            qsched = QSCHED0 if ci == 0 else QSCHED1
                            qk_c = ([(sched, wkt, k_lo, k_hi, bk_t),
                                     (qsched, wqt, q_lo, q_hi, bq_t)])
                            for sch, wt, lo, hi, bias in qk_c:
                                if mt not in sch:
                                    continue
                                qo = 512 * sch[mt]
                                qw = min(512, N - qo)
                                ps_q = finp.tile([128, 512], F32,
                                                 name="qps",
                                                 tag=f"f{mt % 2}")
                                for c2 in range(2):
                                    nc.tensor.matmul(
                                        ps_q[:, 0:qw], wt[c2][:],
                                        xf[c2][:, qo:qo + qw],
                                        start=(c2 == 0), stop=(c2 == 1))
                                qk_bias_split(lo, hi, ps_q, bias, qo, qw)
                        for j in pv_sched.get(mt, ()):
                            emit_pv(j, first=(j == 1))
                        if mt == 3 and pending is not None:
                            emit_proj(*pending)
                            pending = None
                        if mt == NMT - 1:
                            # u0..u3 cover m-tiles 0..15; start the
                            # denominator accumulation while the last exp
                            # is still in flight.
                            for i in range(2):
                                sums_hs[i] = finp.tile(
                                    [1, 512], F32, name="sums_h",
                                    tag=f"f{i}")
                                for uj in range(4):
                                    nc.tensor.matmul(
                                        sums_hs[i][:, 0:512], ones_bf[:],
                                        usum[uj][:, 512 * i:512 * (i + 1)],
                                        start=(uj == 0), stop=False)
                        if mt % 2 == 1 and mt < NMT - 1:
                            k = mt // 2
                            t = workp.tile([128, 1024], BF16,
                                           name=f"t{k}", tag=f"t{k}",
                                           bufs=1)
                            eng = nc.gpsimd if k in gps_t else nc.vector
                            eng.tensor_tensor(
                                t[:], pexp[mt - 1][:], pexp[mt][:],
                                OP.add)
                            tsum[k] = t
                            emit_tree(k)
                    for j in pv_left:
                        emit_pv(j, first=(j == 1))

                    def make_finish(xxt_c=xxt, pexp=pexp, sums_hs=sums_hs,
                                    r_out=r_ts, gps_t=gps_t):
                        def fin():
                            # finish: t8, denominators, normalize,
                            # bias+relu. Only the t8 ones-matmul waits on
                            # the trailing exp.
                            t8 = workp.tile([128, 1024], BF16,
                                            name="t8", tag="t8", bufs=1)
                            eng = nc.gpsimd if 8 in gps_t else nc.vector
                            eng.tensor_tensor(
                                t8[:], pexp[16][:], pexp[17][:], OP.add)
                            for i in range(2):
                                # bank i holds heads i (cols 0:256) and
                                # i+2 (256:512), matching the tree-tile
                                # column halves; the finish reads both as
                                # full [128,512] tiles.
                                nc.tensor.matmul(
                                    sums_hs[i][:, 0:512], ones_bf[:],
                                    t8[:, 512 * i:512 * (i + 1)],
                                    start=False, stop=True)
                                s_inv = workp.tile([1, 512], F32,
                                                   name="s_inv",
                                                   tag=f"s_inv{i}")
                                nc.vector.reciprocal_approx_fast(
                                    s_inv[:, 0:512], sums_hs[i][:, 0:512])
                                bc = workp.tile([128, 512], F32,
                                                name="inv_bc",
                                                tag=f"inv_bc{i}")
                                nc.gpsimd.partition_broadcast(
                                    bc[:, 0:512], s_inv[:, 0:512])
                                t_p = workp.tile([128, 512], F32,
                                                 name="t_p", tag=f"t_p{i}")
                                nc.vector.tensor_tensor(
                                    t_p[:], xxt_c[i][:], bc[:], OP.mult)
                                r_p = rp.tile([128, 512], FP16,
                                              name=f"r{i}", tag=f"r{i}")
                                for hh in range(2):
                                    h = 2 * hh + i
                                    nc.vector.tensor_scalar(
                                        r_p[:, 256 * hh:256 * hh + JW],
                                        t_p[:, 256 * hh:256 * hh + JW],
                                        bv_t[h][:], 0.0, OP.add, OP.max)
                                    r_out[h] = r_p[:, 256 * hh:256 * hh + JW]
                        return fin

                    fin_prev = make_finish()
                    pending = (joff, r_ts)
                fin_prev()
                emit_proj(*pending)
    nc.compile()
    return nc


def _prep_inputs(x, wq, gq, bq, wk, gk, bk, wv, gv, bv, wp, gp, bp):
    """Fold BN scales into weights; build the 8 per-core input maps."""
    rs = np.float32(1.0 / np.sqrt(np.float32(1.0) + np.float32(EPS)))
    sq = (gq * rs).astype(np.float32)
    sk = (gk * rs).astype(np.float32)
    sv = (gv * rs).astype(np.float32)
    sp = (gp * rs).astype(np.float32)
    wq_f = (wq * sq[:, None]).astype(np.float16)
    wk_f = (wk * sk[:, None]).astype(np.float16)
    wv_f = (wv * sv[:, None]).astype(np.float16)
    wp_f = (wp * sp[:, None]).astype(np.float16)

    xf = np.ascontiguousarray(x.reshape(B, C, N).astype(np.float16))
    consts = np.zeros((128, 128 + J), dtype=np.float32)
    consts[:, 0:128] = 1.0
    import ml_dtypes
    ones_bf = np.ones((128, 1), dtype=ml_dtypes.bfloat16)
    in_maps = []
    for core in range(8):
        b, g = core // 2, core % 2
        qs = slice(128 * g, 128 * (g + 1))       # q/k rows for this head group
        vs = slice(512 * g, 512 * (g + 1))       # v rows / p cols for this group
        in_maps.append({
            "x_in": xf[b],
            "wqt": np.ascontiguousarray(wq_f[qs, :].T),
            "wkt": np.ascontiguousarray(wk_f[qs, :].T),
            "wvt": np.ascontiguousarray(wv_f[vs, :].T),
            "wpt": np.ascontiguousarray(wp_f[:, vs].T),
            "bq": np.ascontiguousarray(bq[qs].astype(np.float32)[:, None]),
            "bk": np.ascontiguousarray(bk[qs].astype(np.float32)[:, None]),
            "bv": np.ascontiguousarray(bv[vs].astype(np.float32)[:, None]),
            "consts": consts,
            "ones_bf": ones_bf,
        })
    return in_maps


def kernel(**inputs):
    if "nc" not in _CACHE:
        _CACHE["nc"] = _build_program()
    nc = _CACHE["nc"]

    in_maps = _prep_inputs(**{k: np.asarray(v) for k, v in inputs.items()})
    res = run_bass_kernel_spmd(nc, in_maps, list(range(8)))
    _CACHE["last_results"] = res

    bp = np.asarray(inputs["bp"]).astype(np.float32)
    out = np.empty((B, C, H, W), dtype=np.float32)
    for b in range(B):
        acc = res.results[2 * b]["outp"] + res.results[2 * b + 1]["outp"]
        acc = acc + bp[:, None]
        out[b] = acc.reshape(C, H, W)
    return out
